# revision 1
# baseline (speedup 1.0000x reference)
"""Distributed causal self-attention kernel for one TRN2 chip (8 NeuronCores).

Self-contained: accepts the FULL inputs of reference.setup_inputs(),
shards internally (tensor-parallel over heads: core c computes heads
(2c, 2c+1) for both batches), runs a Bass/Tile kernel SPMD on cores 0-7
with one 8-core AllToAll to reshard head-split -> token-split before the
output projection, and gathers the full [2, 2048, 1024] output.

Compiled graph is cached at module level; first call compiles, later
calls just execute.
"""

import numpy as np
import ml_dtypes
import concourse.bass as bass
import concourse.bacc as bacc
import concourse.tile as tile
import concourse.mybir as mybir

F32 = mybir.dt.float32
F32R = mybir.dt.float32r
BF16 = mybir.dt.bfloat16
Exp = mybir.ActivationFunctionType.Exp

B, T, C, H, HS = 2, 2048, 1024, 16, 64
NCORES = 8
TLOC = 512         # tokens per core after A2A
NKC = C // 128     # contraction tiles
NJT = T // 128     # key tiles per batch
NQB = T // 512     # query blocks per batch
SCALE = 1.0 / np.sqrt(HS)
import os as _os
_flag = lambda name, default: bool(int(_os.environ.get(name, str(int(default)))))
ATT_DT = BF16   # attention matmul operand dtype (F32R or BF16)
PROBE_EXP2X = _flag("PROBE_EXP2X", False)   # issue every exp twice (ACT-load probe)
PROBE_AV2X = _flag("PROBE_AV2X", False)    # issue every AV matmul twice (PE-load probe)
PROBE_NO_NORM = _flag("PROBE_NO_NORM", False)  # skip recip/broadcast/normalize (timing probe)
PROBE_NO_TRI = _flag("PROBE_NO_TRI", False)   # skip triangle masks (timing probe)
ST_SINGLE = _flag("ST_SINGLE", False)      # single-jt [128,512] S^T psum tiles, 6-deep rotation
PT_BUFS = int(_os.environ.get("PT_BUFS", "32"))           # pt pool depth
INTERLEAVE_B = _flag("INTERLEAVE_B", False)   # qb-major loop order (batches interleaved)
PROBE_DMA2X = _flag("PROBE_DMA2X", False)    # duplicate xtb loads (DMA-bound probe)
SPLIT_DMA = _flag("SPLIT_DMA", False)      # spread DMAs across SP and Act HWDGE queues


def build_nc(timeline=False, repeat=1, phases=("qkv", "attn", "a2a", "proj")):
    nc = bacc.Bacc("TRN2", target_bir_lowering=False, debug=False,
                   num_devices=1 if timeline else NCORES)
    xtb_d = nc.dram_tensor("xtb", [C, B * T], BF16, kind="ExternalInput")
    wqk_d = nc.dram_tensor("wqk", [C, 256], BF16, kind="ExternalInput")
    wvb_d = nc.dram_tensor("wvb", [C, 130], BF16, kind="ExternalInput")
    bvb_d = nc.dram_tensor("bvb", [1, 130], BF16, kind="ExternalInput")
    bqk_d = nc.dram_tensor("bqk", [128, 2], F32, kind="ExternalInput")
    wp_d = nc.dram_tensor("wp", [C, C], BF16, kind="ExternalInput")
    bp_d = nc.dram_tensor("bp", [128, 8], F32, kind="ExternalInput")
    out_d = nc.dram_tensor("out", [C, TLOC], F32, kind="ExternalOutput")
    # a2a chunk per dest core: 2 heads x (64 y rows + 1 denom row) = 130 rows

    with tile.TileContext(nc) as tc:
        for _rep in range(repeat):
            _body(nc, tc, xtb_d, wqk_d, wvb_d, bvb_d, bqk_d, wp_d, bp_d,
                  out_d, timeline=timeline, phases=phases)
    nc.compile()
    return nc


def _body(nc, tc, xtb_d, wqk_d, wvb_d, bvb_d, bqk_d, wp_d, bp_d, out_d,
          timeline=False, phases=("qkv", "attn", "a2a", "proj")):
    with (
        tc.tile_pool(name="pers", bufs=1) as pers,
        tc.tile_pool(name="dram", bufs=1, space="DRAM") as dram,
    ):
        a2a_in = dram.tile([NCORES * 130, TLOC], BF16, name="a2a_in")
        a2a_out = dram.tile([NCORES * 130, TLOC], BF16, name="a2a_out")

        wqk = pers.tile([128, NKC, 256], BF16, name="wqk")
        wvb = pers.tile([128, NKC, 130], BF16, name="wvb")
        bvb = pers.tile([1, 130], BF16, name="bvb")
        bqk = pers.tile([128, 2], F32, name="bqk")
        bp = pers.tile([128, 8], F32, name="bp")
        onesb = pers.tile([1, 128], BF16, name="onesb")
        wrm = pers.tile([1, 1], F32, name="wrm")
        tri = pers.tile([128, 128], F32, name="tri")
        qt = [[pers.tile([128, 512], ATT_DT, name=f"qt_{b}_{qb}")
               for qb in range(NQB)] for b in range(B)]
        kt = [[pers.tile([128, 512], ATT_DT, name=f"kt_{b}_{qb}")
               for qb in range(NQB)] for b in range(B)]
        va = [[pers.tile([128, 130], ATT_DT, name=f"va_{b}_{jt}")
               for jt in range(NJT)] for b in range(B)]
        # xts lives in the per-rep outer pool at a stable SBUF address, so
        # the next rep's x loads only WAR against this rep's QKV phase and
        # stream in during attn/a2a/proj
        xts = [pers.tile([128, B * T], BF16, name=f"xts_{kc}")
               for kc in range(NKC)]

        nc.sync.dma_start(
            out=wqk[:], in_=wqk_d.ap().rearrange("(kc p) m -> p kc m", p=128))
        nc.sync.dma_start(
            out=wvb[:], in_=wvb_d.ap().rearrange("(kc p) m -> p kc m", p=128))
        nc.sync.dma_start(out=bvb[:], in_=bvb_d[:])
        nc.sync.dma_start(out=bqk[:], in_=bqk_d[:])
        nc.sync.dma_start(out=bp[:], in_=bp_d[:])
        nc.vector.memset(onesb[:], 1.0)
        nc.vector.memset(wrm[:], 0.0)
        # warm the exp table set early
        nc.scalar.activation(wrm[:], wrm[:], Exp)
        # tri[j, q] = 1 where j <= q else 0
        nc.gpsimd.memset(tri[:], 0.0)
        nc.gpsimd.affine_select(
            out=tri[:], in_=tri[:],
            compare_op=mybir.AluOpType.is_gt, fill=1.0,
            base=0, pattern=[[-1, 128]], channel_multiplier=1,
        )

        # ---------------- Phase 1: QKV ----------------
        if "qkv" not in phases:
            return
        with (
            tc.tile_pool(name="p1", bufs=1) as p1,
            tc.tile_pool(name="ps1", bufs=1, space="PSUM") as ps1,
        ):
            # token-sliced loads on both HWDGE queues: the first (b, qb)
            # group is ready after 1 MB instead of the full 8 MB
            for half in range(8):
                sl = slice(512 * half, 512 * (half + 1))
                for kc in range(NKC):
                    eng = nc.sync if (kc % 2 == 0) else nc.scalar
                    eng.dma_start(out=xts[kc][:, sl],
                                  in_=xtb_d[128 * kc:128 * (kc + 1), sl])
            xt = [[[xts[kc][:, T * b + 512 * qb:T * b + 512 * (qb + 1)]
                    for qb in range(NQB)] for kc in range(NKC)]
                  for b in range(B)]

            for b in range(B):
                for qb in range(NQB):
                    for m in range(2):      # 0: Q, 1: K
                        qk_ps = ps1.tile([128, 512], F32, tag="qk", bufs=4,
                                         name=f"qkps_{m}_{b}_{qb}")
                        for kc in range(NKC):
                            nc.tensor.matmul(
                                qk_ps[:],
                                wqk[:, kc, 128 * m:128 * (m + 1)],
                                xt[b][kc][qb],
                                start=(kc == 0), stop=(kc == NKC - 1))
                        dst = (qt if m == 0 else kt)[b][qb]
                        nc.scalar.activation(dst[:], qk_ps[:],
                                             mybir.ActivationFunctionType.Identity,
                                             bias=bqk[:, m:m + 1])
                    for o in range(4):
                        tt = 4 * qb + o
                        v_ps = ps1.tile([128, 130], F32, tag="v", bufs=4,
                                        name=f"vps_{b}_{tt}")
                        for kc in range(NKC):
                            nc.tensor.matmul(
                                v_ps[:],
                                xts[kc][:, T * b + 512 * qb + 128 * o:
                                        T * b + 512 * qb + 128 * (o + 1)],
                                wvb[:, kc, :],
                                start=(kc == 0), stop=False)
                        nc.tensor.matmul(v_ps[:], onesb[:], bvb[:],
                                         start=False, stop=True)
                        nc.scalar.activation(va[b][tt][:], v_ps[:],
                                             mybir.ActivationFunctionType.Copy)

        # ---------------- Phase 2: attention ----------------
        if "attn" not in phases:
            return
        with (
            tc.tile_pool(name="p2", bufs=1) as p2,
            tc.tile_pool(name="ps2", bufs=1, space="PSUM") as ps2,
        ):
            ynall = p2.tile([65, 16, TLOC], BF16, name="ynall")
            # qb-descending, b-interleaved: adjacent blocks have similar
            # size, so AV-vs-S pacing stays smooth at block boundaries
            bq_order = [(b, qb) for qb in range(NQB - 1, -1, -1)
                        for b in range(B)]

            def emit_s_unit(b, qb, jg, h, ptl):
                """S matmuls + exp (+ diag tri) for one [128,1024] st tile."""
                stp = ps2.tile([128, 1024], F32, tag="st", bufs=3,
                               name=f"st_{b}_{qb}_{jg}_{h}")
                offs = [128 * (2 * jg + jj - 4 * qb)
                        if 2 * jg + jj >= 4 * qb else 0 for jj in range(2)]
                for jj in range(2):
                    jt = 2 * jg + jj
                    nc.tensor.matmul(
                        stp[:, 512 * jj + offs[jj]:512 * (jj + 1)],
                        kt[b][jt // 4][64 * h:64 * (h + 1),
                                       128 * (jt % 4):128 * (jt % 4 + 1)],
                        qt[b][qb][64 * h:64 * (h + 1), offs[jj]:512],
                        start=True, stop=True,
                        tile_position=(64 * h, 0))
                ptile = p2.tile([128, 1024], ATT_DT, tag="pt", bufs=PT_BUFS,
                                name=f"pt_{b}_{qb}_{jg}_{h}")
                if offs[0] == 0 and offs[1] == 0:
                    nc.scalar.activation(ptile[:], stp[:], Exp,
                                         scale=float(SCALE))
                else:
                    for jj in range(2):
                        sl = slice(512 * jj + offs[jj], 512 * (jj + 1))
                        nc.scalar.activation(ptile[:, sl], stp[:, sl], Exp,
                                             scale=float(SCALE))
                for jj in range(2):
                    jt = 2 * jg + jj
                    if jt >= 4 * qb:
                        o = jt - 4 * qb
                        sl = slice(512 * jj + 128 * o,
                                   512 * jj + 128 * (o + 1))
                        nc.vector.tensor_mul(ptile[:, sl], ptile[:, sl],
                                             tri[:])
                ptl[(h, jg)] = ptile

            def av_units(b, qb, ptl):
                """Yield thunks: the AV accumulation chain + psum->sbuf copy."""
                njt = 4 * (qb + 1)
                for h in range(2):
                    yps = ps2.tile([65, 512], F32, tag="yt", bufs=2,
                                   name=f"yps_{b}_{qb}_{h}")
                    for jt in range(njt):
                        off = 128 * (jt - 4 * qb) if jt >= 4 * qb else 0
                        base = 512 * (jt % 2)
                        yield lambda h=h, jt=jt, off=off, base=base, \
                            yps=yps: nc.tensor.matmul(
                                yps[:, off:512],
                                va[b][jt][:, 65 * h:65 * (h + 1)],
                                ptl[(h, jt // 2)][:, base + off:base + 512],
                                start=(jt == 0), stop=(jt == njt - 1))
                    j = 2 * (4 * b + qb) + h
                    yn = ynall[:, j, :]
                    def fin(yn=yn, yps=yps, j=j):
                        nc.vector.tensor_copy(yn, yps[:])
                        nc.sync.dma_start(out=a2a_in[65 * j:65 * (j + 1), :],
                                          in_=yn)
                    yield fin

            # Software pipeline: while emitting block n's S/exp units, drain
            # block n-1's AV units between them so PE fills ACT-bound stalls.
            av_pending = []
            for b, qb in bq_order:
                njg = 2 * (qb + 1)
                units = [(jg, h) for jg in range(njg) for h in range(2)]
                n_slots = max(1, len(units) - 3)
                per_slot = (len(av_pending) + n_slots - 1) // n_slots
                ptl = {}
                for i, (jg, h) in enumerate(units):
                    if i >= 3:
                        for _ in range(per_slot):
                            if av_pending:
                                av_pending.pop(0)()
                    emit_s_unit(b, qb, jg, h, ptl)
                while av_pending:
                    av_pending.pop(0)()
                av_pending = list(av_units(b, qb, ptl))
            for u in av_pending:
                u()

        # ---------------- Phase 3: A2A + c_proj ----------------
        if "a2a" not in phases:
            return
        if timeline:
            # stand-in for the A2A so the single-core cost model runs
            nc.sync.dma_start(out=a2a_out[:], in_=a2a_in[:])
        else:
            nc.gpsimd.collective_compute(
                "AllToAll", mybir.AluOpType.bypass,
                replica_groups=[list(range(NCORES))],
                ins=[a2a_in.opt()], outs=[a2a_out.opt()])

        if "proj" not in phases:
            return
        with (
            tc.tile_pool(name="p3", bufs=1) as p3,
            tc.tile_pool(name="ps3", bufs=1, space="PSUM") as ps3,
        ):
            wp = p3.tile([128, NKC, C], BF16, name="wp")
            (nc.scalar if SPLIT_DMA else nc.sync).dma_start(
                out=wp[:],
                in_=wp_d.ap().rearrange("(kc p) m -> p kc m", p=128))
            # a2a_out rows: 65*j + r for slab j = 2*g + h (g = src core, h =
            # head parity) -> head index j; r<64: y channel dim, r==64: the
            # softmax denominator row.
            v65 = a2a_out[:].rearrange("(j x) t -> x j t", x=65)
            vj = a2a_out[:].rearrange("(g h x) t -> x g h t", h=2, x=65)
            yls = p3.tile([128, NKC, TLOC], BF16, name="yls")
            nc.sync.dma_start(
                out=yls[0:64, :, :],
                in_=vj[0:64, :, 0:1, :].rearrange("x g one t -> x g (one t)"))
            nc.sync.dma_start(
                out=yls[64:128, :, :],
                in_=vj[0:64, :, 1:2, :].rearrange("x g one t -> x g (one t)"))
            den = p3.tile([16, TLOC], BF16, name="den")
            nc.sync.dma_start(
                out=den[:],
                in_=v65[64:65].rearrange("one j t -> (one j) t"))
            # den[j] = denominator of head j for this core's tokens
            rden_f = p3.tile([16, TLOC], F32, name="rden_f")
            nc.vector.reciprocal(rden_f[:], den[:])
            rden = p3.tile([16, TLOC], BF16, name="rden")
            nc.scalar.activation(rden[:], rden_f[:],
                                 mybir.ActivationFunctionType.Copy)
            # sel[j, kc, 64h:64h+64] = 1 iff j == 2*kc+h: maps rden rows onto
            # the 128 channels of kc's tile via one rank-16 matmul per kc.
            sel = p3.tile([16, NKC, 128], BF16, name="sel")
            nc.gpsimd.memset(sel[:], 1.0)
            nc.gpsimd.affine_select(
                out=sel[:].rearrange("j g (h c) -> j g h c", h=2),
                in_=sel[:].rearrange("j g (h c) -> j g h c", h=2),
                compare_op=mybir.AluOpType.is_equal, fill=0.0,
                base=0, pattern=[[-2, NKC], [-1, 2], [0, 64]],
                channel_multiplier=1)
            ynm = p3.tile([128, NKC, TLOC], BF16, name="ynm")
            for kc in range(NKC):
                rbc = ps3.tile([128, TLOC], F32, tag="rbc", bufs=2,
                               name=f"rbc_{kc}")
                nc.tensor.matmul(rbc[:], sel[:, kc, :], rden[:],
                                 start=True, stop=True)
                nc.vector.tensor_mul(ynm[:, kc, :], yls[:, kc, :], rbc[:])
            osball = p3.tile([128, 8, TLOC], F32, name="osball")
            for m in range(8):
                pj = ps3.tile([128, TLOC], F32, tag="pj", bufs=4,
                              name=f"pj_{m}")
                for kc in range(NKC):
                    nc.tensor.matmul(
                        pj[:],
                        wp[:, kc, 128 * m:128 * (m + 1)],
                        ynm[:, kc, :],
                        start=(kc == 0), stop=(kc == NKC - 1))
                osb = osball[:, m, :]
                nc.vector.tensor_scalar_add(osb, pj[:], bp[:, m:m + 1])
            nc.sync.dma_start(
                out=out_d.ap().rearrange("(m p) t -> p m t", p=128),
                in_=osball[:])


def prep_inputs(x, W_attn, b_attn, W_proj, b_proj):
    """Full inputs -> list of 8 per-core input dicts."""
    x = np.asarray(x, dtype=np.float32)
    W_attn = np.asarray(W_attn, dtype=np.float32)
    b_attn = np.asarray(b_attn, dtype=np.float32)
    W_proj = np.asarray(W_proj, dtype=np.float32)
    b_proj = np.asarray(b_proj, dtype=np.float32)
    bf16 = ml_dtypes.bfloat16
    xtb = np.ascontiguousarray(
        np.concatenate([x[0].T, x[1].T], axis=1).astype(bf16))
    in_maps = []
    for c in range(NCORES):
        h0, h1 = 2 * c, 2 * c + 1
        qcols = np.r_[64 * h0:64 * h0 + 64, 64 * h1:64 * h1 + 64]
        kcols = C + qcols
        vcols = 2 * C + qcols
        wqk = np.concatenate([W_attn[:, qcols], W_attn[:, kcols]], axis=1)
        wvb = np.zeros((C, 130), np.float32)
        wvb[:, 0:64] = W_attn[:, vcols[0:64]]
        wvb[:, 65:129] = W_attn[:, vcols[64:128]]
        bvb = np.zeros((1, 130), np.float32)
        bvb[0, 0:64] = b_attn[vcols[0:64]]
        bvb[0, 65:129] = b_attn[vcols[64:128]]
        bvb[0, 64] = 1.0
        bvb[0, 129] = 1.0
        bqk = np.stack([b_attn[qcols], b_attn[kcols]], axis=1)
        in_maps.append({
            "xtb": xtb,
            "wqk": np.ascontiguousarray(wqk.astype(bf16)),
            "wvb": np.ascontiguousarray(wvb.astype(bf16)),
            "bvb": np.ascontiguousarray(bvb.astype(bf16)),
            "bqk": np.ascontiguousarray(bqk.astype(np.float32)),
            "wp": np.ascontiguousarray(W_proj.astype(bf16)),
            "bp": np.ascontiguousarray(
                b_proj.reshape(8, 128).T.astype(np.float32)),
        })
    return in_maps


def assemble(results):
    """Per-core {'out': [C, TLOC]} -> full [B, T, C]."""
    out = np.empty((B, T, C), dtype=np.float32)
    for c in range(NCORES):
        b, g = c // 4, c % 4
        out[b, TLOC * g:TLOC * (g + 1), :] = results[c]["out"].T
    return out


_CACHE = {}


def kernel(x, W_attn, b_attn, W_proj, b_proj):
    from concourse.bass_utils import run_bass_kernel_spmd

    if "nc" not in _CACHE:
        _CACHE["nc"] = build_nc()
    nc = _CACHE["nc"]
    in_maps = prep_inputs(x, W_attn, b_attn, W_proj, b_proj)
    res = run_bass_kernel_spmd(nc, in_maps, core_ids=list(range(NCORES)))
    return assemble(res.results)



# revision 8
# speedup vs baseline: 1.6071x; 1.6071x over previous
"""Distributed causal self-attention kernel for one TRN2 chip (8 NeuronCores).

Self-contained: accepts the FULL inputs of reference.setup_inputs(),
shards internally (tensor-parallel over heads: core c computes heads
(2c, 2c+1) for both batches), runs a Bass/Tile kernel SPMD on cores 0-7
with one 8-core AllToAll to reshard head-split -> token-split before the
output projection, and gathers the full [2, 2048, 1024] output.

Structure: QKV and attention are FUSED into 8 rounds, one per
(batch, 512-query-block) in qb-ascending order, so attention for round r
depends only on QKV of rounds <= r. This hides the 8 MB x load (which
arrives one 1 MB token-slice at a time) under compute instead of
stalling the PE for ~20 us up front. Each round emits its QKV matmul
chains and S/exp units while draining the previous round's AV chains
between them (software pipeline); the last round self-drains. PSUM->SBUF
copies ride the DVE so the ACT engine does (almost) only exp.

Compiled graph is cached at module level; first call compiles, later
calls just execute.
"""

import numpy as np
import ml_dtypes
import concourse.bass as bass
import concourse.bacc as bacc
import concourse.tile as tile
import concourse.mybir as mybir

F32 = mybir.dt.float32
BF16 = mybir.dt.bfloat16
Exp = mybir.ActivationFunctionType.Exp

B, T, C, H, HS = 2, 2048, 1024, 16, 64
NCORES = 8
TLOC = 512         # tokens per core after A2A
NKC = C // 128     # contraction tiles
NJT = T // 128     # key tiles per batch
NQB = T // 512     # query blocks per batch
SCALE = 1.0 / np.sqrt(HS)
ATT_DT = BF16

import os as _os
PT_BUFS = int(_os.environ.get("PT_BUFS", "24"))
ST_BUFS = int(_os.environ.get("ST_BUFS", "2"))
MM_BUFS = int(_os.environ.get("MM_BUFS", "2"))
AV_LAG = int(_os.environ.get("AV_LAG", "1"))


def build_nc(timeline=False, repeat=1, phases=("fused", "a2a", "proj")):
    nc = bacc.Bacc("TRN2", target_bir_lowering=False, debug=False,
                   num_devices=1 if timeline else NCORES)
    xtb_d = nc.dram_tensor("xtb", [C, B * T], BF16, kind="ExternalInput")
    wqk_d = nc.dram_tensor("wqk", [C, 256], BF16, kind="ExternalInput")
    wvb_d = nc.dram_tensor("wvb", [C, 130], BF16, kind="ExternalInput")
    bvb_d = nc.dram_tensor("bvb", [1, 130], BF16, kind="ExternalInput")
    bqk_d = nc.dram_tensor("bqk", [128, 2], F32, kind="ExternalInput")
    wp_d = nc.dram_tensor("wp", [C, C], BF16, kind="ExternalInput")
    bp_d = nc.dram_tensor("bp", [128, 8], F32, kind="ExternalInput")
    out_d = nc.dram_tensor("out", [C, TLOC], F32, kind="ExternalOutput")
    # a2a chunk per dest core: 2 heads x (64 y rows + 1 denom row) = 130 rows

    with tile.TileContext(nc) as tc:
        for _rep in range(repeat):
            _body(nc, tc, xtb_d, wqk_d, wvb_d, bvb_d, bqk_d, wp_d, bp_d,
                  out_d, timeline=timeline, phases=phases)
    nc.compile()
    return nc


def _body(nc, tc, xtb_d, wqk_d, wvb_d, bvb_d, bqk_d, wp_d, bp_d, out_d,
          timeline=False, phases=("fused", "a2a", "proj")):
    with (
        tc.tile_pool(name="pers", bufs=1) as pers,
        tc.tile_pool(name="dram", bufs=1, space="DRAM") as dram,
    ):
        a2a_in = dram.tile([NCORES * 130, TLOC], BF16, name="a2a_in")
        a2a_out = dram.tile([NCORES * 130, TLOC], BF16, name="a2a_out")

        wqk = pers.tile([128, NKC, 256], BF16, name="wqk")
        wvb = pers.tile([128, NKC, 130], BF16, name="wvb")
        bvb = pers.tile([1, 130], BF16, name="bvb")
        bqk = pers.tile([128, 2], F32, name="bqk")
        bp = pers.tile([128, 8], F32, name="bp")
        wp = pers.tile([128, NKC, C], BF16, name="wp")
        onesb = pers.tile([1, 128], BF16, name="onesb")
        wrm = pers.tile([1, 1], F32, name="wrm")
        tri = pers.tile([128, 128], F32, name="tri")
        qt = [[pers.tile([128, 512], ATT_DT, name=f"qt_{b}_{qb}")
               for qb in range(NQB)] for b in range(B)]
        kt = [[pers.tile([128, 512], ATT_DT, name=f"kt_{b}_{qb}")
               for qb in range(NQB)] for b in range(B)]
        va = [[pers.tile([128, 130], ATT_DT, name=f"va_{b}_{jt}")
               for jt in range(NJT)] for b in range(B)]
        ynall = pers.tile([65, 16, TLOC], BF16, name="ynall")
        # xts lives at a stable SBUF address so the next rep's x loads only
        # WAR against this rep's round-r readers and stream in early
        xts_all = pers.tile([128, NKC, B * T], BF16, name="xts")
        xts = [xts_all[:, kc, :] for kc in range(NKC)]

        sel = pers.tile([16, NKC, 128], BF16, name="sel")
        nc.sync.dma_start(
            out=wqk[:], in_=wqk_d.ap().rearrange("(kc p) m -> p kc m", p=128))
        nc.scalar.dma_start(
            out=wvb[:], in_=wvb_d.ap().rearrange("(kc p) m -> p kc m", p=128))
        nc.scalar.dma_start(out=bvb[:], in_=bvb_d[:])
        nc.scalar.dma_start(out=bqk[:], in_=bqk_d[:])
        nc.scalar.dma_start(out=bp[:], in_=bp_d[:])
        nc.vector.memset(onesb[:], 1.0)
        nc.vector.memset(wrm[:], 0.0)
        # warm the exp table set early
        nc.scalar.activation(wrm[:], wrm[:], Exp)
        # tri[j, q] = 1 where j <= q else 0
        nc.gpsimd.memset(tri[:], 0.0)
        nc.gpsimd.affine_select(
            out=tri[:], in_=tri[:],
            compare_op=mybir.AluOpType.is_gt, fill=1.0,
            base=0, pattern=[[-1, 128]], channel_multiplier=1,
        )
        # sel[j, kc, 64h:64h+64] = 1 iff j == 2*kc+h: maps the per-head
        # softmax denominators onto the 128 channels of proj input tile kc
        # via one rank-16 matmul per kc (input-independent -> built here)
        nc.gpsimd.memset(sel[:], 1.0)
        nc.gpsimd.affine_select(
            out=sel[:].rearrange("j g (h c) -> j g h c", h=2),
            in_=sel[:].rearrange("j g (h c) -> j g h c", h=2),
            compare_op=mybir.AluOpType.is_equal, fill=0.0,
            base=0, pattern=[[-2, NKC], [-1, 2], [0, 64]],
            channel_multiplier=1)

        if "fused" not in phases:
            return

        # rounds in (qb, b) ascending order; x token-slices stream in the
        # same order, one merged DMA per slice. All x rides the sync queue:
        # DMAs dispatched from the scalar queue hold the ACT sequencer,
        # which must stay free to issue exps once attention starts.
        rounds = [(b, qb) for qb in range(NQB) for b in range(B)]
        xtb_v = xtb_d.ap().rearrange("(kc p) t -> p kc t", p=128)
        for b, qb in rounds:
            half = 4 * b + qb
            sl = slice(512 * half, 512 * (half + 1))
            nc.sync.dma_start(out=xts_all[:, :, sl], in_=xtb_v[:, :, sl])
        # prefetch the proj weight behind the x slices (needed ~100us later)
        nc.scalar.dma_start(
            out=wp[:], in_=wp_d.ap().rearrange("(kc p) m -> p kc m", p=128))

        with (
            tc.tile_pool(name="p2", bufs=1) as p2,
            tc.tile_pool(name="ps2", bufs=1, space="PSUM") as ps2,
        ):
            def emit_qkv_units(b, qb):
                """Yield thunks: Q/K chains, then V chains for this round."""
                xt = [xts[kc][:, T * b + 512 * qb:T * b + 512 * (qb + 1)]
                      for kc in range(NKC)]
                for m in range(2):      # 0: Q, 1: K
                    def qk(m=m, xt=xt):
                        qk_ps = ps2.tile([128, 512], F32, tag="mm",
                                         bufs=MM_BUFS, name=f"qkps_{m}_{b}_{qb}")
                        for kc in range(NKC):
                            nc.tensor.matmul(
                                qk_ps[:],
                                wqk[:, kc, 128 * m:128 * (m + 1)],
                                xt[kc],
                                start=(kc == 0), stop=(kc == NKC - 1))
                        dst = (qt if m == 0 else kt)[b][qb]
                        nc.vector.tensor_scalar_add(dst[:], qk_ps[:],
                                                    bqk[:, m:m + 1])
                    yield qk
                for o in range(4):
                    def vv(o=o):
                        tt = 4 * qb + o
                        v_ps = ps2.tile([128, 512], F32, tag="mm",
                                        bufs=MM_BUFS, name=f"vps_{b}_{tt}")
                        vp = v_ps[:, 0:130]
                        for kc in range(NKC):
                            nc.tensor.matmul(
                                vp,
                                xts[kc][:, T * b + 512 * qb + 128 * o:
                                        T * b + 512 * qb + 128 * (o + 1)],
                                wvb[:, kc, :],
                                start=(kc == 0), stop=False)
                        nc.tensor.matmul(vp, onesb[:], bvb[:],
                                         start=False, stop=True)
                        nc.vector.tensor_copy(va[b][tt][:], vp)
                    yield vv

            def emit_s_unit(b, qb, jg, h, ptl):
                """S matmuls + exp (+ diag tri) for one [128,1024] st tile."""
                stp = ps2.tile([128, 1024], F32, tag="st", bufs=ST_BUFS,
                               name=f"st_{b}_{qb}_{jg}_{h}")
                offs = [128 * (2 * jg + jj - 4 * qb)
                        if 2 * jg + jj >= 4 * qb else 0 for jj in range(2)]
                for jj in range(2):
                    jt = 2 * jg + jj
                    nc.tensor.matmul(
                        stp[:, 512 * jj + offs[jj]:512 * (jj + 1)],
                        kt[b][jt // 4][64 * h:64 * (h + 1),
                                       128 * (jt % 4):128 * (jt % 4 + 1)],
                        qt[b][qb][64 * h:64 * (h + 1), offs[jj]:512],
                        start=True, stop=True,
                        tile_position=(64 * h, 0))
                ptile = p2.tile([128, 1024], ATT_DT, tag="pt", bufs=PT_BUFS,
                                name=f"pt_{b}_{qb}_{jg}_{h}")
                if offs[0] == 0 and offs[1] == 0:
                    nc.scalar.activation(ptile[:], stp[:], Exp,
                                         scale=float(SCALE))
                else:
                    for jj in range(2):
                        sl = slice(512 * jj + offs[jj], 512 * (jj + 1))
                        nc.scalar.activation(ptile[:, sl], stp[:, sl], Exp,
                                             scale=float(SCALE))
                for jj in range(2):
                    jt = 2 * jg + jj
                    if jt >= 4 * qb:
                        o = jt - 4 * qb
                        sl = slice(512 * jj + 128 * o,
                                   512 * jj + 128 * (o + 1))
                        nc.vector.tensor_mul(ptile[:, sl], ptile[:, sl],
                                             tri[:])
                ptl[(h, jg)] = ptile

            def av_units(b, qb, ptl):
                """Yield (min_pair, thunk): AV chain steps + psum->sbuf copy
                + (h=1) the merged a2a_in store. min_pair = the S-unit pair
                index the step needs (for self-drain gating)."""
                njt = 4 * (qb + 1)
                for h in range(2):
                    yps = ps2.tile([65, 512], F32, tag="yt", bufs=2,
                                   name=f"yps_{b}_{qb}_{h}")
                    for jt in range(njt):
                        off = 128 * (jt - 4 * qb) if jt >= 4 * qb else 0
                        base = 512 * (jt % 2)
                        yield jt // 2, (lambda h=h, jt=jt, off=off, base=base,
                                        yps=yps: nc.tensor.matmul(
                                            yps[:, off:512],
                                            va[b][jt][:, 65 * h:65 * (h + 1)],
                                            ptl[(h, jt // 2)][:, base + off:
                                                              base + 512],
                                            start=(jt == 0),
                                            stop=(jt == njt - 1)))
                    i = 4 * b + qb
                    j = 2 * i + h

                    def fin(yps=yps, j=j, i=i, h=h):
                        nc.vector.tensor_copy(ynall[:, j, :], yps[:])
                        if h == 1:
                            nc.sync.dma_start(
                                out=a2a_in[130 * i:130 * (i + 1), :],
                                in_=ynall[:, 2 * i:2 * i + 2, :])
                    yield njt // 2 - 1, fin

            # Software pipeline: while emitting round r's QKV chains and
            # S/exp units, drain round r-1's AV units between them so the
            # PE fills ACT-bound stalls. The last round additionally
            # self-drains its own AV (gated on which S pairs have been
            # emitted) so it isn't a serial tail after the final exp.
            av_pending = []   # list of (min_pair or None, thunk)
            pairs_done = -1   # S pairs emitted in the current round

            def drain(limit=None):
                n = 0
                while av_pending and (limit is None or n < limit):
                    mp, thunk = av_pending[0]
                    if mp is not None and mp > pairs_done:
                        break
                    av_pending.pop(0)
                    thunk()
                    n += 1

            for rnd, (b, qb) in enumerate(rounds):
                last = rnd == len(rounds) - 1
                njg = 2 * (qb + 1)
                s_units = [(jg, h) for jg in range(njg) for h in range(2)]
                emitters = list(emit_qkv_units(b, qb))
                n_slots = len(emitters) + len(s_units)
                per_slot = (len(av_pending) + n_slots - 1) // max(1, n_slots)
                ptl = {}
                pairs_done = -1

                for e in emitters:
                    e()
                    drain(per_slot)
                if last:
                    # queue this round's own AV behind the previous round's,
                    # gated on S-pair availability
                    av_pending.extend(av_units(b, qb, ptl))
                for jg, h in s_units:
                    emit_s_unit(b, qb, jg, h, ptl)
                    if h == 1:
                        pairs_done = jg - AV_LAG
                    drain(per_slot if not last else None)
                pairs_done = njg
                drain()
                if not last:
                    av_pending = [(None, th)
                                  for _, th in av_units(b, qb, ptl)]

        # ---------------- A2A + c_proj ----------------
        if "a2a" not in phases:
            return
        if timeline:
            # stand-in for the A2A so the single-core cost model runs
            nc.sync.dma_start(out=a2a_out[:], in_=a2a_in[:])
        else:
            nc.gpsimd.collective_compute(
                "AllToAll", mybir.AluOpType.bypass,
                replica_groups=[list(range(NCORES))],
                ins=[a2a_in.opt()], outs=[a2a_out.opt()])

        if "proj" not in phases:
            return
        with (
            tc.tile_pool(name="p3", bufs=1) as p3,
            tc.tile_pool(name="ps3", bufs=1, space="PSUM") as ps3,
        ):
            # a2a_out chunk g (from src core g) rows: 2*x + h for x = y
            # channel dim 0..64, h = head parity (the merged per-unit store
            # writes ynall[:, 2i:2i+2, :] partition-major, interleaving the
            # two heads). x==64 is the softmax denominator row; head index
            # j = 2*g + h.
            vj = a2a_out[:].rearrange("(g x h) t -> x g h t", h=2, x=65)
            den = p3.tile([16, TLOC], BF16, name="den")
            nc.sync.dma_start(
                out=den[:],
                in_=a2a_out[:].rearrange("(g r) t -> g r t",
                                         r=130)[:, 128:130, :])
            yls = p3.tile([128, NKC, TLOC], BF16, name="yls")
            nc.sync.dma_start(
                out=yls[0:64, :, :],
                in_=vj[0:64, :, 0:1, :].rearrange("x g one t -> x g (one t)"))
            nc.scalar.dma_start(
                out=yls[64:128, :, :],
                in_=vj[0:64, :, 1:2, :].rearrange("x g one t -> x g (one t)"))
            # den[j] = denominator of head j for this core's tokens
            rden_f = p3.tile([16, TLOC], F32, name="rden_f")
            nc.vector.reciprocal(rden_f[:], den[:])
            rden = p3.tile([16, TLOC], BF16, name="rden")
            nc.vector.tensor_copy(rden[:], rden_f[:])
            ynm = p3.tile([128, NKC, TLOC], BF16, name="ynm")
            for kc in range(NKC):
                rbc = ps3.tile([128, TLOC], F32, tag="rbc", bufs=2,
                               name=f"rbc_{kc}")
                nc.tensor.matmul(rbc[:], sel[:, kc, :], rden[:],
                                 start=True, stop=True)
                nc.vector.tensor_mul(ynm[:, kc, :], yls[:, kc, :], rbc[:])
            osball = p3.tile([128, 8, TLOC], F32, name="osball")
            for m in range(8):
                pj = ps3.tile([128, TLOC], F32, tag="pj", bufs=4,
                              name=f"pj_{m}")
                for kc in range(NKC):
                    nc.tensor.matmul(
                        pj[:],
                        wp[:, kc, 128 * m:128 * (m + 1)],
                        ynm[:, kc, :],
                        start=(kc == 0), stop=(kc == NKC - 1))
                osb = osball[:, m, :]
                nc.vector.tensor_scalar_add(osb, pj[:], bp[:, m:m + 1])
                eng = nc.sync if m % 2 == 0 else nc.scalar
                eng.dma_start(
                    out=out_d[:].rearrange("(m p) t -> p m t", p=128)
                    [:, m:m + 1, :].rearrange("p one t -> p (one t)"),
                    in_=osb)


def prep_inputs(x, W_attn, b_attn, W_proj, b_proj):
    """Full inputs -> list of 8 per-core input dicts."""
    x = np.asarray(x, dtype=np.float32)
    W_attn = np.asarray(W_attn, dtype=np.float32)
    b_attn = np.asarray(b_attn, dtype=np.float32)
    W_proj = np.asarray(W_proj, dtype=np.float32)
    b_proj = np.asarray(b_proj, dtype=np.float32)
    bf16 = ml_dtypes.bfloat16
    xtb = np.ascontiguousarray(
        np.concatenate([x[0].T, x[1].T], axis=1).astype(bf16))
    in_maps = []
    for c in range(NCORES):
        h0, h1 = 2 * c, 2 * c + 1
        qcols = np.r_[64 * h0:64 * h0 + 64, 64 * h1:64 * h1 + 64]
        kcols = C + qcols
        vcols = 2 * C + qcols
        wqk = np.concatenate([W_attn[:, qcols], W_attn[:, kcols]], axis=1)
        wvb = np.zeros((C, 130), np.float32)
        wvb[:, 0:64] = W_attn[:, vcols[0:64]]
        wvb[:, 65:129] = W_attn[:, vcols[64:128]]
        bvb = np.zeros((1, 130), np.float32)
        bvb[0, 0:64] = b_attn[vcols[0:64]]
        bvb[0, 65:129] = b_attn[vcols[64:128]]
        bvb[0, 64] = 1.0
        bvb[0, 129] = 1.0
        bqk = np.stack([b_attn[qcols], b_attn[kcols]], axis=1)
        in_maps.append({
            "xtb": xtb,
            "wqk": np.ascontiguousarray(wqk.astype(bf16)),
            "wvb": np.ascontiguousarray(wvb.astype(bf16)),
            "bvb": np.ascontiguousarray(bvb.astype(bf16)),
            "bqk": np.ascontiguousarray(bqk.astype(np.float32)),
            "wp": np.ascontiguousarray(W_proj.astype(bf16)),
            "bp": np.ascontiguousarray(
                b_proj.reshape(8, 128).T.astype(np.float32)),
        })
    return in_maps


def assemble(results):
    """Per-core {'out': [C, TLOC]} -> full [B, T, C]."""
    out = np.empty((B, T, C), dtype=np.float32)
    for c in range(NCORES):
        b, g = c // 4, c % 4
        out[b, TLOC * g:TLOC * (g + 1), :] = results[c]["out"].T
    return out


_CACHE = {}


def kernel(x, W_attn, b_attn, W_proj, b_proj):
    from concourse.bass_utils import run_bass_kernel_spmd

    if "nc" not in _CACHE:
        _CACHE["nc"] = build_nc()
    nc = _CACHE["nc"]
    in_maps = prep_inputs(x, W_attn, b_attn, W_proj, b_proj)
    res = run_bass_kernel_spmd(nc, in_maps, core_ids=list(range(NCORES)))
    return assemble(res.results)
